# revision 27
# baseline (speedup 1.0000x reference)
"""Trainium2 Bass kernel for nn_CNNEmbedding: char-CNN word embedding.

Reference computation (per flattened word, NW=16384 words):
  x = emb[char_ids]                       # [16, 64]
  for w in 1..6: y_w = conv1d(x.T, W_w, 'wide' pad) ; f_w = max_t tanh(y_w + b_w)
  f = concat(f_w)                         # [525]
  out[word_pos, word_batch] = f           # [256, 64, 525]

Kernel strategy (8 NeuronCores, data-parallel over words, 2048 words/core):
  - tanh is monotonic => max-pool BEFORE bias+tanh.
  - embedding lookup via one-hot matmul: ids broadcast to 128 partitions,
    VectorE tensor_scalar(is_equal) against a per-partition iota builds the
    one-hot [vocab-half, cols]; two accumulating matmuls against the
    (d-duplicated) embedding table produce x directly as
    [128 partitions (d dup), word-cols] in PSUM. The id plane is DENSE
    (16 cols/word); ScalarE scatters PSUM->SBUF into the strided 21-col
    x-plane (top half as-is, bottom half shifted left one column so a
    single K=128 conv matmul computes TWO taps). Guard cols stay zero
    from a one-time memset.
  - convs packed into 5 PSUM-row tiles so multiple chains share both the
    matmul passes and the reduce:
      A: k6[0:75] + k5[0:53]    3 passes  window [0,21)
      B: k6[75:150] + k5[53:106] 3 passes window [0,21)
      C: k5[106:125] + k4        3 passes window [1,21)
      D: k3 + k1                 2 passes window [0,18)
      E: k2                      1 pass   window [0,17)
    12 passes total (vs 15 for per-chain tiles). Each pass streams 21
    contiguous cols/word; out-of-window cols read only the zero guard so
    they evaluate to exactly 0, making a single full-window reduce_max
    per tile valid (max(y,0) flips only when a word's true max < 0 --
    probability ~2^-16, negligible vs the 2e-2 gate).
  - max over time: ONE VectorE reduce_max per (tile, psum-unit) covering
    all packed rows -> packed feats tiles.
  - ScalarE fused bias+tanh on packed feats; DMA-xbar transpose
    [rows,128]->[128,rows] per word-block; SWDGE cast-DMA (bf16->fp32)
    writes each chain's channel slice straight to DRAM.
"""

import os
import numpy as np
import ml_dtypes

# ---- problem constants (hardcoded; kernel.py must be self-contained) ----
B = 64
WORDS = 256
NW = B * WORDS          # 16384
LMAX = 16
V = 256
D = 64
KS = [1, 2, 3, 4, 5, 6]
CS = [25, 50, 75, 100, 125, 150]
CTOT = sum(CS)          # 525

NCORES = 8
NWC = NW // NCORES      # 2048 words per core
GW = 512                # words per group
NGROUP = NWC // GW      # 4
S = 21                  # word stride in x-plane (16 chars + 5 shared zero pad)
DOFF = 5                # first char col within a word block
NIDXG = ((S * GW + DOFF + 127) // 128) * 128   # 10880 cols per group x-plane
DCOLS = LMAX * GW       # 8192 dense id cols per group
ECH = 512               # embed matmul chunk (one PSUM bank)

OUT_OFF = np.concatenate([[0], np.cumsum(CS)]).tolist()

_BF16 = ml_dtypes.bfloat16

_CACHE = {}

# ---- packed conv tiles -------------------------------------------------
# member: (ki, clo, chi, r0, m)  rows [r0, r0+chi-clo), col-map offset m
# tile: dict(members, sigmas, win0, win)
TILES = [
    dict(members=[(4, 0, 53, 0, 1), (5, 0, 75, 53, 0)],
         sigmas=[-5, -3, -1], win0=0, win=21),
    dict(members=[(4, 53, 106, 0, 1), (5, 75, 150, 53, 0)],
         sigmas=[-5, -3, -1], win0=0, win=21),
    dict(members=[(3, 0, 100, 0, 2), (4, 106, 125, 100, 1)],
         sigmas=[-5, -3, -1], win0=1, win=20),
    dict(members=[(0, 0, 25, 0, 0), (2, 0, 75, 25, 0)],
         sigmas=[-2, 0], win0=0, win=18),
    dict(members=[(1, 0, 50, 0, 0)],
         sigmas=[-1], win0=0, win=17),
]
NTILE = len(TILES)
for _t in TILES:
    _t["rows"] = max(r0 + chi - clo for (_, clo, chi, r0, _) in _t["members"])

# junk cols inside a tile's reduce window for a member whose valid window
# is narrower: (tile -> (row0, row1, jcol0, jcol1)); these evaluate to
# exactly 0 (they read only the zero guard), which would clip the max at 0,
# so a ScalarE splat overwrites them with -1e4 before the reduce.
JUNK = {0: (0, 53, 0, 1), 1: (0, 53, 0, 1),
        2: (0, 100, 1, 2), 3: (0, 25, 16, 18)}

NPASS = sum(len(t["sigmas"]) for t in TILES)   # 12
# wall layout: block i -> cols [128*i, 128*(i+1)), in tile-then-sigma order
_WOFF = {}
_off = 0
for _ti, _t in enumerate(TILES):
    for _s in _t["sigmas"]:
        _WOFF[(_ti, _s)] = _off
        _off += 128
WALL_COLS = _off                               # 12*128 = 1536


def _tap_assign():
    """For each (tile, member, sigma): (dt_top or None, dt_bot or None)."""
    asg = {}
    for ti, t in enumerate(TILES):
        for mi, (ki, clo, chi, r0, m) in enumerate(t["members"]):
            w = KS[ki]
            need = set(range(w))
            for s in t["sigmas"]:
                dt_top = s + m + w - 1
                dt_bot = dt_top + 1
                top = dt_top if dt_top in need else None
                if top is not None:
                    need.discard(dt_top)
                bot = dt_bot if dt_bot in need else None
                if bot is not None:
                    need.discard(dt_bot)
                asg[(ti, mi, s)] = (top, bot)
            assert not need, (ti, mi, need)
    return asg


TAP_ASG = _tap_assign()

# output slices: (tile, col0_in_tr, C, out_off)
OUT_SLICES = []
for _ti, _t in enumerate(TILES):
    for (_ki, _clo, _chi, _r0, _m) in _t["members"]:
        OUT_SLICES.append((_ti, _r0, _chi - _clo, OUT_OFF[_ki] + _clo))


def _build_program():
    from contextlib import ExitStack

    import concourse.mybir as mybir
    import concourse.tile as tile
    from concourse import bacc

    dt = mybir.dt
    nc = bacc.Bacc("TRN2", target_bir_lowering=False, debug=False,
                   num_devices=NCORES)

    idsd = nc.dram_tensor("ids", [1, NGROUP * DCOLS], dt.bfloat16,
                          kind="ExternalInput").ap()
    etab = nc.dram_tensor("etab", [128, 256], dt.bfloat16,
                          kind="ExternalInput").ap()
    iotad = nc.dram_tensor("iota", [128, 2], dt.float32,
                           kind="ExternalInput").ap()
    wall = nc.dram_tensor("wall", [128, WALL_COLS], dt.bfloat16,
                          kind="ExternalInput").ap()
    biasd = nc.dram_tensor("bias", [128, NTILE], dt.float32,
                           kind="ExternalInput").ap()
    fout = nc.dram_tensor("f", [NTILE * 128, NWC], dt.bfloat16,
                          kind="ExternalOutput").ap()

    import concourse.bass as bass

    with tile.TileContext(nc) as tc, ExitStack() as ctx:
        singles = ctx.enter_context(tc.tile_pool(name="singles", bufs=1))
        idsp = ctx.enter_context(tc.tile_pool(name="idsp", bufs=2))
        ohp = ctx.enter_context(tc.tile_pool(name="ohp", bufs=4))
        xpp = ctx.enter_context(tc.tile_pool(name="xpp", bufs=2))
        psp = ctx.enter_context(tc.tile_pool(name="psp", bufs=2, space="PSUM"))
        psep = ctx.enter_context(tc.tile_pool(name="psep", bufs=2,
                                              space="PSUM"))

        etab_sb = singles.tile([128, 256], dt.bfloat16, tag="etab")
        nc.sync.dma_start(out=etab_sb, in_=etab)
        iota_sb = singles.tile([128, 2], dt.float32, tag="iota")
        nc.sync.dma_start(out=iota_sb, in_=iotad)
        wall_sb = singles.tile([128, WALL_COLS], dt.bfloat16, tag="wall")
        nc.sync.dma_start(out=wall_sb, in_=wall)
        bias_sb = singles.tile([128, NTILE], dt.float32, tag="bias")
        nc.sync.dma_start(out=bias_sb, in_=biasd)

        feats = [
            singles.tile([128, NWC], dt.bfloat16, tag=f"feats{i}",
                         name=f"feats{i}")
            for i in range(NTILE)
        ]

        # two persistent x-planes (guard cols stay zero after this memset)
        xps = [singles.tile([128, NIDXG], dt.bfloat16, tag=f"xp{i}",
                            name=f"xp{i}") for i in range(2)]

        def memset_guards():
            # only the inter-word guard cols and the tail need zeroing:
            # top rows need [21n, 21n+5), shifted bottom rows [21n-1, 21n+4)
            for i in range(2):
                xp = xps[i]
                nc.gpsimd.memset(xp[:, 0:DOFF], 0.0)
                nc.gpsimd.memset(
                    xp[:, S - 1: S - 1 + S * (GW - 1)].rearrange(
                        "p (n t) -> p n t", t=S)[:, :, 0:6], 0.0)
                nc.gpsimd.memset(xp[:, S * GW - 1: NIDXG], 0.0)

        def emit_embed_group(g):
            """Embed group g into x-plane xps[g % 2] via one-hot matmuls.
            Returns emission thunks (one per 2-chunk pse pair)."""
            ids_sb = idsp.tile([128, DCOLS], dt.bfloat16, tag="ids",
                               name=f"ids{g}")
            bcast = bass.AP(
                tensor=idsd.tensor,
                offset=g * DCOLS,
                ap=[[0, 128], [1, DCOLS]],
            )
            nc.gpsimd.dma_start(out=ids_sb, in_=bcast)
            xp = xps[g % 2]

            thunks = []
            nch = DCOLS // (2 * ECH)   # 8 pse pairs of 2*512 dense cols

            def mk(tci):
                def emit():
                    c0 = tci * 2 * ECH
                    ohs = []
                    for h in range(2):
                        oh = ohp.tile([128, 2 * ECH], dt.bfloat16, tag="oh",
                                      name=f"oh{g}_{tci}_{h}")
                        nc.vector.tensor_scalar(
                            out=oh,
                            in0=ids_sb[:, c0: c0 + 2 * ECH],
                            scalar1=iota_sb[:, h:h + 1],
                            scalar2=None,
                            op0=mybir.AluOpType.is_equal,
                        )
                        ohs.append(oh)
                    for j in range(0, 2 * ECH, ECH):
                        pse = psep.tile([128, ECH], dt.float32, tag="pse",
                                        name=f"pse{g}_{tci}_{j}")
                        for h in range(2):
                            nc.tensor.matmul(
                                pse,
                                lhsT=etab_sb[:, h * 128:(h + 1) * 128],
                                rhs=ohs[h][:, j:j + ECH],
                                start=(h == 0),
                                stop=(h == 1),
                            )
                        # scatter into strided x-plane: 32 words per chunk
                        w0 = (c0 + j) // LMAX
                        nw = ECH // LMAX
                        src = pse.rearrange("p (n t) -> p n t", t=LMAX)
                        top = xp[0:64, S * w0 + DOFF:
                                 S * w0 + DOFF + S * nw].rearrange(
                            "p (n t) -> p n t", t=S)[:, :, 0:LMAX]
                        bot = xp[64:128, S * w0 + DOFF - 1:
                                 S * w0 + DOFF - 1 + S * nw].rearrange(
                            "p (n t) -> p n t", t=S)[:, :, 0:LMAX]
                        nc.scalar.copy(out=top, in_=src[0:64])
                        nc.scalar.copy(out=bot, in_=src[64:128])
                return emit

            for tci in range(nch):
                thunks.append(mk(tci))
            return xp, thunks

        def emit_tile_unit(xp, g, ti, tg):
            """Matmuls + packed reduce for one (conv tile, psum unit)."""
            t = TILES[ti]
            rows = t["rows"]
            ps = psp.tile([128, 3, 512], dt.float32, tag="ps",
                          name=f"ps{g}_{ti}_{tg[0][0]}")
            nsig = len(t["sigmas"])
            win0, win = t["win0"], t["win"]
            # narrow tiles stream only their reduce window's cols per word
            narrow = win < 19
            sw = win if narrow else S
            for si, sg in enumerate(t["sigmas"]):
                boff = _WOFF[(ti, sg)]
                o = DOFF + sg
                for j, (cn0, cnw) in enumerate(tg):
                    rhs = xp[0:128, S * cn0 + o: S * cn0 + o + S * cnw]
                    if narrow:
                        rhs = rhs.rearrange("k (n t) -> k n t",
                                            t=S)[:, :, 0:win]
                    nc.tensor.matmul(
                        ps[:, j, 0:cnw * sw],
                        lhsT=wall_sb[0:128, boff: boff + 128],
                        rhs=rhs,
                        start=(si == 0),
                        stop=(si == nsig - 1),
                    )
            r0 = 0
            while r0 < len(tg):
                r1 = r0
                while r1 < len(tg) and tg[r1][1] == tg[r0][1]:
                    r1 += 1
                na, nwd = r1 - r0, tg[r0][1]
                rw0 = 0 if narrow else win0
                if ti in JUNK:
                    jr0, jr1, jc0, jc1 = JUNK[ti]
                    jv = ps[jr0:jr1, r0:r1, 0:nwd * sw].rearrange(
                        "c a (n t) -> c a n t", t=sw)[:, :, :, jc0:jc1]
                    nc.scalar.activation(
                        out=jv, in_=jv,
                        func=mybir.ActivationFunctionType.Copy,
                        bias=-1.0e4, scale=0.0)
                src = ps[0:rows, r0:r1, 0:nwd * sw].rearrange(
                    "c a (n t) -> c a n t", t=sw)[:, :, :, rw0:rw0 + win]
                w0 = g * GW + tg[r0][0]
                dst = feats[ti][0:rows, w0: w0 + na * nwd].rearrange(
                    "c (a n) -> c a n", n=nwd)
                nc.vector.reduce_max(out=dst, in_=src,
                                     axis=mybir.AxisListType.X)
                r0 = r1

        def emit_output_group(g):
            """bias+tanh per packed tile, then DMA the packed rows out;
            the host unpacks rows->channels and transposes."""
            w0 = g * GW
            for ti, t in enumerate(TILES):
                nc.scalar.activation(
                    out=feats[ti][0:t["rows"], w0:w0 + GW],
                    in_=feats[ti][0:t["rows"], w0:w0 + GW],
                    func=mybir.ActivationFunctionType.Tanh,
                    bias=bias_sb[0:t["rows"], ti:ti + 1],
                )
                nc.sync.dma_start(
                    out=fout[ti * 128: ti * 128 + t["rows"], w0:w0 + GW],
                    in_=feats[ti][0:t["rows"], w0:w0 + GW])

        # psum units: chunks of 24 words (504 cols), paired into 2-bank units
        wpb = 512 // S
        chunks = []
        n0 = 0
        while n0 < GW:
            chunks.append((n0, min(wpb, GW - n0)))
            n0 += wpb
        units = [tuple(chunks[i:i + 3]) for i in range(0, len(chunks), 3)]

        # group-0 embed thunks are emitted lazily, right before the first
        # conv unit that needs their words (thunk i covers words 32i..32i+32)
        xp_cur, cthunks = emit_embed_group(0)   # issues g0 ids DMA first
        memset_guards()
        cdone = 0

        for g in range(NGROUP):
            items = [(ti, tg) for tg in units for ti in range(NTILE)]
            if g + 1 < NGROUP:
                xp_next, nthunks = emit_embed_group(g + 1)
            else:
                xp_next, nthunks = None, []
            ne, ni = len(nthunks), len(items)
            ti_ = 0
            for k, (ti, tg) in enumerate(items):
                wend = tg[-1][0] + tg[-1][1]      # last word this unit reads
                need = min(len(cthunks), -(-wend // 64))
                while cdone < need:
                    cthunks[cdone]()
                    cdone += 1
                emit_tile_unit(xp_cur, g, ti, tg)
                want = (k + 1) * ne // ni
                while ti_ < want:
                    nthunks[ti_]()
                    ti_ += 1
            while cdone < len(cthunks):
                cthunks[cdone]()
                cdone += 1
            while ti_ < ne:
                nthunks[ti_]()
                ti_ += 1
            if g > 0:
                emit_output_group(g - 1)
            xp_cur = xp_next
            cthunks, cdone = nthunks, len(nthunks)
        emit_output_group(NGROUP - 1)

    nc.compile()
    return nc


def _host_consts(emb, Ws, bs):
    e = emb.astype(_BF16)
    etab = np.zeros((128, 256), dtype=_BF16)
    for h in range(2):
        etab[:, h * 128: h * 128 + 64] = e[h * 128:(h + 1) * 128, :]
        etab[:, h * 128 + 64: h * 128 + 128] = e[h * 128:(h + 1) * 128, :]

    iota = np.zeros((128, 2), dtype=np.float32)
    iota[:, 0] = np.arange(128)
    iota[:, 1] = np.arange(128, 256)

    wall = np.zeros((128, WALL_COLS), dtype=_BF16)
    for ti, t in enumerate(TILES):
        for mi, (ki, clo, chi, r0, m) in enumerate(t["members"]):
            Wb = Ws[ki].astype(np.float32)
            c = chi - clo
            for sg in t["sigmas"]:
                boff = _WOFF[(ti, sg)]
                dt_top, dt_bot = TAP_ASG[(ti, mi, sg)]
                if dt_top is not None:
                    wall[0:64, boff + r0: boff + r0 + c] = (
                        Wb[clo:chi, :, dt_top].T.astype(_BF16))
                if dt_bot is not None:
                    wall[64:128, boff + r0: boff + r0 + c] = (
                        Wb[clo:chi, :, dt_bot].T.astype(_BF16))

    bias = np.zeros((128, NTILE), dtype=np.float32)
    for ti, t in enumerate(TILES):
        for (ki, clo, chi, r0, m) in t["members"]:
            bias[r0: r0 + chi - clo, ti] = bs[ki][clo:chi]
    return etab, iota, wall, bias


def _host_ids(char_ids_core):
    """Per-core dense id plane [1, NGROUP*DCOLS] bf16."""
    return (char_ids_core.astype(np.float32)
            .reshape(1, -1).astype(_BF16))


def kernel(**inputs):
    import jax

    jax.devices()  # boot the axon PJRT backend
    from concourse.bass_utils import run_bass_kernel_spmd

    char_ids = np.asarray(inputs["char_ids"], dtype=np.int32)
    word_pos = np.asarray(inputs["word_pos"], dtype=np.int64)
    word_batch = np.asarray(inputs["word_batch"], dtype=np.int64)
    emb = np.asarray(inputs["emb"], dtype=np.float32)
    Ws = [np.asarray(inputs[f"W{i+1}"], dtype=np.float32) for i in range(6)]
    bs = [np.asarray(inputs[f"b{i+1}"], dtype=np.float32) for i in range(6)]

    if "nc" not in _CACHE:
        _CACHE["nc"] = _build_program()
    nc = _CACHE["nc"]

    etab, iota, wall, bias = _host_consts(emb, Ws, bs)
    in_maps = []
    for c in range(NCORES):
        in_maps.append({
            "ids": _host_ids(char_ids[c * NWC:(c + 1) * NWC]),
            "etab": etab,
            "iota": iota,
            "wall": wall,
            "bias": bias,
        })

    core_ids = list(range(NCORES))
    trace = bool(os.environ.get("KERNEL_TRACE"))
    res = run_bass_kernel_spmd(nc, in_maps, core_ids, trace=trace)
    if trace:
        _CACHE["last_exec_time_ns"] = res.exec_time_ns

    f_full = np.empty((NW, CTOT), dtype=np.float32)
    for c in core_ids:
        fb = np.asarray(res.results[c]["f"])      # [NTILE*128, NWC] bf16
        dst = f_full[c * NWC:(c + 1) * NWC]
        for (ti, r0, C, oo) in OUT_SLICES:
            dst[:, oo:oo + C] = fb[ti * 128 + r0: ti * 128 + r0 + C, :].T

    out = np.zeros((WORDS, B, CTOT), dtype=np.float32)
    out[word_pos, word_batch] = f_full
    return out


# revision 28
# speedup vs baseline: 1.1080x; 1.1080x over previous
"""Trainium2 Bass kernel for nn_CNNEmbedding: char-CNN word embedding.

Reference computation (per flattened word, NW=16384 words):
  x = emb[char_ids]                       # [16, 64]
  for w in 1..6: y_w = conv1d(x.T, W_w, 'wide' pad) ; f_w = max_t tanh(y_w + b_w)
  f = concat(f_w)                         # [525]
  out[word_pos, word_batch] = f           # [256, 64, 525]

Kernel strategy (8 NeuronCores, data-parallel over words, 2048 words/core):
  - tanh is monotonic => max-pool BEFORE bias+tanh.
  - embedding lookup via one-hot matmul: ids broadcast to 128 partitions,
    VectorE tensor_scalar(is_equal) against a per-partition iota builds the
    one-hot [vocab-half, cols]; two accumulating matmuls against the
    (d-duplicated) embedding table produce x directly as
    [128 partitions (d dup), word-cols] in PSUM. The id plane is DENSE
    (16 cols/word); ScalarE scatters PSUM->SBUF into the strided 21-col
    x-plane (top half as-is, bottom half shifted left one column so a
    single K=128 conv matmul computes TWO taps). Guard cols stay zero
    from a one-time memset.
  - convs packed into 5 PSUM-row tiles so multiple chains share both the
    matmul passes and the reduce:
      A: k6[0:75] + k5[0:53]    3 passes  window [0,21)
      B: k6[75:150] + k5[53:106] 3 passes window [0,21)
      C: k5[106:125] + k4        3 passes window [1,21)
      D: k3 + k1                 2 passes window [0,18)
      E: k2                      1 pass   window [0,17)
    12 passes total (vs 15 for per-chain tiles). Each pass streams 21
    contiguous cols/word; out-of-window cols read only the zero guard so
    they evaluate to exactly 0, making a single full-window reduce_max
    per tile valid (max(y,0) flips only when a word's true max < 0 --
    probability ~2^-16, negligible vs the 2e-2 gate).
  - max over time: ONE VectorE reduce_max per (tile, psum-unit) covering
    all packed rows -> packed feats tiles.
  - ScalarE fused bias+tanh on packed feats; DMA-xbar transpose
    [rows,128]->[128,rows] per word-block; SWDGE cast-DMA (bf16->fp32)
    writes each chain's channel slice straight to DRAM.
"""

import os
import numpy as np
import ml_dtypes

# ---- problem constants (hardcoded; kernel.py must be self-contained) ----
B = 64
WORDS = 256
NW = B * WORDS          # 16384
LMAX = 16
V = 256
D = 64
KS = [1, 2, 3, 4, 5, 6]
CS = [25, 50, 75, 100, 125, 150]
CTOT = sum(CS)          # 525

NCORES = 8
NWC = NW // NCORES      # 2048 words per core
GW = 512                # words per group
NGROUP = NWC // GW      # 4
S = 21                  # word stride in x-plane (16 chars + 5 shared zero pad)
DOFF = 5                # first char col within a word block
NIDXG = ((S * GW + DOFF + 127) // 128) * 128   # 10880 cols per group x-plane
DCOLS = LMAX * GW       # 8192 dense id cols per group
ECH = 512               # embed matmul chunk (one PSUM bank)

OUT_OFF = np.concatenate([[0], np.cumsum(CS)]).tolist()

_BF16 = ml_dtypes.bfloat16

_CACHE = {}

# ---- packed conv tiles -------------------------------------------------
# member: (ki, clo, chi, r0, m)  rows [r0, r0+chi-clo), col-map offset m
# tile: dict(members, sigmas, win0, win)
TILES = [
    dict(members=[(4, 0, 53, 0, 1), (5, 0, 75, 53, 0)],
         sigmas=[-5, -3, -1], win0=0, win=21),
    dict(members=[(4, 53, 106, 0, 1), (5, 75, 150, 53, 0)],
         sigmas=[-5, -3, -1], win0=0, win=21),
    dict(members=[(3, 0, 100, 0, 2), (4, 106, 125, 100, 1)],
         sigmas=[-5, -3, -1], win0=1, win=20),
    dict(members=[(0, 0, 25, 0, 0), (2, 0, 75, 25, 0)],
         sigmas=[-2, 0], win0=0, win=18),
    dict(members=[(1, 0, 50, 0, 0)],
         sigmas=[-1], win0=0, win=17),
]
NTILE = len(TILES)
for _t in TILES:
    _t["rows"] = max(r0 + chi - clo for (_, clo, chi, r0, _) in _t["members"])

# junk cols inside a tile's reduce window for a member whose valid window
# is narrower: (tile -> (row0, row1, jcol0, jcol1)); these evaluate to
# exactly 0 (they read only the zero guard), which would clip the max at 0,
# so a ScalarE splat overwrites them with -1e4 before the reduce.
JUNK = {0: (0, 53, 0, 1), 1: (0, 53, 0, 1),
        2: (0, 100, 1, 2), 3: (0, 25, 16, 18)}

NPASS = sum(len(t["sigmas"]) for t in TILES)   # 12
# wall layout: block i -> cols [128*i, 128*(i+1)), in tile-then-sigma order
_WOFF = {}
_off = 0
for _ti, _t in enumerate(TILES):
    for _s in _t["sigmas"]:
        _WOFF[(_ti, _s)] = _off
        _off += 128
WALL_COLS = _off                               # 12*128 = 1536


def _tap_assign():
    """For each (tile, member, sigma): (dt_top or None, dt_bot or None)."""
    asg = {}
    for ti, t in enumerate(TILES):
        for mi, (ki, clo, chi, r0, m) in enumerate(t["members"]):
            w = KS[ki]
            need = set(range(w))
            for s in t["sigmas"]:
                dt_top = s + m + w - 1
                dt_bot = dt_top + 1
                top = dt_top if dt_top in need else None
                if top is not None:
                    need.discard(dt_top)
                bot = dt_bot if dt_bot in need else None
                if bot is not None:
                    need.discard(dt_bot)
                asg[(ti, mi, s)] = (top, bot)
            assert not need, (ti, mi, need)
    return asg


TAP_ASG = _tap_assign()

# output slices: (tile, col0_in_tr, C, out_off)
OUT_SLICES = []
for _ti, _t in enumerate(TILES):
    for (_ki, _clo, _chi, _r0, _m) in _t["members"]:
        OUT_SLICES.append((_ti, _r0, _chi - _clo, OUT_OFF[_ki] + _clo))


def _build_program():
    from contextlib import ExitStack

    import concourse.mybir as mybir
    import concourse.tile as tile
    from concourse import bacc

    dt = mybir.dt
    nc = bacc.Bacc("TRN2", target_bir_lowering=False, debug=False,
                   num_devices=NCORES)

    idsd = nc.dram_tensor("ids", [1, NGROUP * DCOLS], dt.bfloat16,
                          kind="ExternalInput").ap()
    etab = nc.dram_tensor("etab", [128, 256], dt.bfloat16,
                          kind="ExternalInput").ap()
    iotad = nc.dram_tensor("iota", [128, 2], dt.float32,
                           kind="ExternalInput").ap()
    wall = nc.dram_tensor("wall", [128, WALL_COLS], dt.bfloat16,
                          kind="ExternalInput").ap()
    biasd = nc.dram_tensor("bias", [128, NTILE], dt.float32,
                           kind="ExternalInput").ap()
    fout = nc.dram_tensor("f", [NTILE * 128, NWC], dt.bfloat16,
                          kind="ExternalOutput").ap()

    import concourse.bass as bass

    with tile.TileContext(nc) as tc, ExitStack() as ctx:
        singles = ctx.enter_context(tc.tile_pool(name="singles", bufs=1))
        idsp = ctx.enter_context(tc.tile_pool(name="idsp", bufs=2))
        ohp = ctx.enter_context(tc.tile_pool(name="ohp", bufs=4))
        xpp = ctx.enter_context(tc.tile_pool(name="xpp", bufs=2))
        psp = ctx.enter_context(tc.tile_pool(name="psp", bufs=3, space="PSUM"))
        psep = ctx.enter_context(tc.tile_pool(name="psep", bufs=2,
                                              space="PSUM"))

        etab_sb = singles.tile([128, 256], dt.bfloat16, tag="etab")
        nc.sync.dma_start(out=etab_sb, in_=etab)
        iota_sb = singles.tile([128, 2], dt.float32, tag="iota")
        nc.sync.dma_start(out=iota_sb, in_=iotad)
        wall_sb = singles.tile([128, WALL_COLS], dt.bfloat16, tag="wall")
        nc.sync.dma_start(out=wall_sb, in_=wall)
        bias_sb = singles.tile([128, NTILE], dt.float32, tag="bias")
        nc.sync.dma_start(out=bias_sb, in_=biasd)

        feats = [
            singles.tile([128, NWC], dt.bfloat16, tag=f"feats{i}",
                         name=f"feats{i}")
            for i in range(NTILE)
        ]

        # two persistent x-planes (guard cols stay zero after this memset)
        xps = [singles.tile([128, NIDXG], dt.bfloat16, tag=f"xp{i}",
                            name=f"xp{i}") for i in range(2)]

        def memset_guards():
            # only the inter-word guard cols and the tail need zeroing:
            # top rows need [21n, 21n+5), shifted bottom rows [21n-1, 21n+4)
            for i in range(2):
                xp = xps[i]
                nc.gpsimd.memset(xp[:, 0:DOFF], 0.0)
                nc.gpsimd.memset(
                    xp[:, S - 1: S - 1 + S * (GW - 1)].rearrange(
                        "p (n t) -> p n t", t=S)[:, :, 0:6], 0.0)
                nc.gpsimd.memset(xp[:, S * GW - 1: NIDXG], 0.0)

        def emit_embed_group(g):
            """Embed group g into x-plane xps[g % 2] via one-hot matmuls.
            Returns emission thunks (one per 2-chunk pse pair)."""
            ids_sb = idsp.tile([128, DCOLS], dt.bfloat16, tag="ids",
                               name=f"ids{g}")
            bcast = bass.AP(
                tensor=idsd.tensor,
                offset=g * DCOLS,
                ap=[[0, 128], [1, DCOLS]],
            )
            nc.gpsimd.dma_start(out=ids_sb, in_=bcast)
            xp = xps[g % 2]

            thunks = []
            nch = DCOLS // (2 * ECH)   # 8 pse pairs of 2*512 dense cols

            def mk(tci):
                def emit():
                    c0 = tci * 2 * ECH
                    ohs = []
                    for h in range(2):
                        oh = ohp.tile([128, 2 * ECH], dt.bfloat16, tag="oh",
                                      name=f"oh{g}_{tci}_{h}")
                        nc.vector.tensor_scalar(
                            out=oh,
                            in0=ids_sb[:, c0: c0 + 2 * ECH],
                            scalar1=iota_sb[:, h:h + 1],
                            scalar2=None,
                            op0=mybir.AluOpType.is_equal,
                        )
                        ohs.append(oh)
                    for j in range(0, 2 * ECH, ECH):
                        pse = psep.tile([128, ECH], dt.float32, tag="pse",
                                        name=f"pse{g}_{tci}_{j}")
                        for h in range(2):
                            nc.tensor.matmul(
                                pse,
                                lhsT=etab_sb[:, h * 128:(h + 1) * 128],
                                rhs=ohs[h][:, j:j + ECH],
                                start=(h == 0),
                                stop=(h == 1),
                            )
                        # scatter into strided x-plane: 32 words per chunk
                        w0 = (c0 + j) // LMAX
                        nw = ECH // LMAX
                        src = pse.rearrange("p (n t) -> p n t", t=LMAX)
                        top = xp[0:64, S * w0 + DOFF:
                                 S * w0 + DOFF + S * nw].rearrange(
                            "p (n t) -> p n t", t=S)[:, :, 0:LMAX]
                        bot = xp[64:128, S * w0 + DOFF - 1:
                                 S * w0 + DOFF - 1 + S * nw].rearrange(
                            "p (n t) -> p n t", t=S)[:, :, 0:LMAX]
                        nc.scalar.copy(out=top, in_=src[0:64])
                        nc.scalar.copy(out=bot, in_=src[64:128])
                return emit

            for tci in range(nch):
                thunks.append(mk(tci))
            return xp, thunks

        def emit_tile_unit(xp, g, ti, tg):
            """Matmuls + packed reduce for one (conv tile, psum unit)."""
            t = TILES[ti]
            rows = t["rows"]
            ps = psp.tile([128, 2, 512], dt.float32, tag="ps",
                          name=f"ps{g}_{ti}_{tg[0][0]}")
            nsig = len(t["sigmas"])
            win0, win = t["win0"], t["win"]
            # narrow tiles stream only their reduce window's cols per word
            narrow = win < 19
            sw = win if narrow else S
            for si, sg in enumerate(t["sigmas"]):
                boff = _WOFF[(ti, sg)]
                o = DOFF + sg
                for j, (cn0, cnw) in enumerate(tg):
                    rhs = xp[0:128, S * cn0 + o: S * cn0 + o + S * cnw]
                    if narrow:
                        rhs = rhs.rearrange("k (n t) -> k n t",
                                            t=S)[:, :, 0:win]
                    nc.tensor.matmul(
                        ps[:, j, 0:cnw * sw],
                        lhsT=wall_sb[0:128, boff: boff + 128],
                        rhs=rhs,
                        start=(si == 0),
                        stop=(si == nsig - 1),
                    )
            r0 = 0
            while r0 < len(tg):
                r1 = r0
                while r1 < len(tg) and tg[r1][1] == tg[r0][1]:
                    r1 += 1
                na, nwd = r1 - r0, tg[r0][1]
                rw0 = 0 if narrow else win0
                if ti in JUNK:
                    jr0, jr1, jc0, jc1 = JUNK[ti]
                    jv = ps[jr0:jr1, r0:r1, 0:nwd * sw].rearrange(
                        "c a (n t) -> c a n t", t=sw)[:, :, :, jc0:jc1]
                    nc.scalar.activation(
                        out=jv, in_=jv,
                        func=mybir.ActivationFunctionType.Copy,
                        bias=-1.0e4, scale=0.0)
                src = ps[0:rows, r0:r1, 0:nwd * sw].rearrange(
                    "c a (n t) -> c a n t", t=sw)[:, :, :, rw0:rw0 + win]
                w0 = g * GW + tg[r0][0]
                dst = feats[ti][0:rows, w0: w0 + na * nwd].rearrange(
                    "c (a n) -> c a n", n=nwd)
                nc.vector.reduce_max(out=dst, in_=src,
                                     axis=mybir.AxisListType.X)
                r0 = r1

        def emit_output_group(g):
            """bias+tanh per packed tile, then DMA the packed rows out;
            the host unpacks rows->channels and transposes."""
            w0 = g * GW
            for ti, t in enumerate(TILES):
                nc.scalar.activation(
                    out=feats[ti][0:t["rows"], w0:w0 + GW],
                    in_=feats[ti][0:t["rows"], w0:w0 + GW],
                    func=mybir.ActivationFunctionType.Tanh,
                    bias=bias_sb[0:t["rows"], ti:ti + 1],
                )
                nc.sync.dma_start(
                    out=fout[ti * 128: ti * 128 + t["rows"], w0:w0 + GW],
                    in_=feats[ti][0:t["rows"], w0:w0 + GW])

        # psum units: chunks of 24 words (504 cols), paired into 2-bank units
        wpb = 512 // S
        chunks = []
        n0 = 0
        while n0 < GW:
            chunks.append((n0, min(wpb, GW - n0)))
            n0 += wpb
        units = [tuple(chunks[i:i + 2]) for i in range(0, len(chunks), 2)]

        # group-0 embed thunks are emitted lazily, right before the first
        # conv unit that needs their words (thunk i covers words 32i..32i+32)
        xp_cur, cthunks = emit_embed_group(0)   # issues g0 ids DMA first
        memset_guards()
        cdone = 0

        for g in range(NGROUP):
            items = [(ti, tg) for tg in units for ti in range(NTILE)]
            if g + 1 < NGROUP:
                xp_next, nthunks = emit_embed_group(g + 1)
            else:
                xp_next, nthunks = None, []
            ne, ni = len(nthunks), len(items)
            ti_ = 0
            for k, (ti, tg) in enumerate(items):
                wend = tg[-1][0] + tg[-1][1]      # last word this unit reads
                need = min(len(cthunks), -(-wend // 64))
                while cdone < need:
                    cthunks[cdone]()
                    cdone += 1
                emit_tile_unit(xp_cur, g, ti, tg)
                want = (k + 1) * ne // ni
                while ti_ < want:
                    nthunks[ti_]()
                    ti_ += 1
            while cdone < len(cthunks):
                cthunks[cdone]()
                cdone += 1
            while ti_ < ne:
                nthunks[ti_]()
                ti_ += 1
            if g > 0:
                emit_output_group(g - 1)
            xp_cur = xp_next
            cthunks, cdone = nthunks, len(nthunks)
        emit_output_group(NGROUP - 1)

    nc.compile()
    return nc


def _host_consts(emb, Ws, bs):
    e = emb.astype(_BF16)
    etab = np.zeros((128, 256), dtype=_BF16)
    for h in range(2):
        etab[:, h * 128: h * 128 + 64] = e[h * 128:(h + 1) * 128, :]
        etab[:, h * 128 + 64: h * 128 + 128] = e[h * 128:(h + 1) * 128, :]

    iota = np.zeros((128, 2), dtype=np.float32)
    iota[:, 0] = np.arange(128)
    iota[:, 1] = np.arange(128, 256)

    wall = np.zeros((128, WALL_COLS), dtype=_BF16)
    for ti, t in enumerate(TILES):
        for mi, (ki, clo, chi, r0, m) in enumerate(t["members"]):
            Wb = Ws[ki].astype(np.float32)
            c = chi - clo
            for sg in t["sigmas"]:
                boff = _WOFF[(ti, sg)]
                dt_top, dt_bot = TAP_ASG[(ti, mi, sg)]
                if dt_top is not None:
                    wall[0:64, boff + r0: boff + r0 + c] = (
                        Wb[clo:chi, :, dt_top].T.astype(_BF16))
                if dt_bot is not None:
                    wall[64:128, boff + r0: boff + r0 + c] = (
                        Wb[clo:chi, :, dt_bot].T.astype(_BF16))

    bias = np.zeros((128, NTILE), dtype=np.float32)
    for ti, t in enumerate(TILES):
        for (ki, clo, chi, r0, m) in t["members"]:
            bias[r0: r0 + chi - clo, ti] = bs[ki][clo:chi]
    return etab, iota, wall, bias


def _host_ids(char_ids_core):
    """Per-core dense id plane [1, NGROUP*DCOLS] bf16."""
    return (char_ids_core.astype(np.float32)
            .reshape(1, -1).astype(_BF16))


def kernel(**inputs):
    import jax

    jax.devices()  # boot the axon PJRT backend
    from concourse.bass_utils import run_bass_kernel_spmd

    char_ids = np.asarray(inputs["char_ids"], dtype=np.int32)
    word_pos = np.asarray(inputs["word_pos"], dtype=np.int64)
    word_batch = np.asarray(inputs["word_batch"], dtype=np.int64)
    emb = np.asarray(inputs["emb"], dtype=np.float32)
    Ws = [np.asarray(inputs[f"W{i+1}"], dtype=np.float32) for i in range(6)]
    bs = [np.asarray(inputs[f"b{i+1}"], dtype=np.float32) for i in range(6)]

    if "nc" not in _CACHE:
        _CACHE["nc"] = _build_program()
    nc = _CACHE["nc"]

    etab, iota, wall, bias = _host_consts(emb, Ws, bs)
    in_maps = []
    for c in range(NCORES):
        in_maps.append({
            "ids": _host_ids(char_ids[c * NWC:(c + 1) * NWC]),
            "etab": etab,
            "iota": iota,
            "wall": wall,
            "bias": bias,
        })

    core_ids = list(range(NCORES))
    trace = bool(os.environ.get("KERNEL_TRACE"))
    res = run_bass_kernel_spmd(nc, in_maps, core_ids, trace=trace)
    if trace:
        _CACHE["last_exec_time_ns"] = res.exec_time_ns

    f_full = np.empty((NW, CTOT), dtype=np.float32)
    for c in core_ids:
        fb = np.asarray(res.results[c]["f"])      # [NTILE*128, NWC] bf16
        dst = f_full[c * NWC:(c + 1) * NWC]
        for (ti, r0, C, oo) in OUT_SLICES:
            dst[:, oo:oo + C] = fb[ti * 128 + r0: ti * 128 + r0 + C, :].T

    out = np.zeros((WORDS, B, CTOT), dtype=np.float32)
    out[word_pos, word_batch] = f_full
    return out


# revision 29
# speedup vs baseline: 1.1617x; 1.0485x over previous
"""Trainium2 Bass kernel for nn_CNNEmbedding: char-CNN word embedding.

Reference computation (per flattened word, NW=16384 words):
  x = emb[char_ids]                       # [16, 64]
  for w in 1..6: y_w = conv1d(x.T, W_w, 'wide' pad) ; f_w = max_t tanh(y_w + b_w)
  f = concat(f_w)                         # [525]
  out[word_pos, word_batch] = f           # [256, 64, 525]

Kernel strategy (8 NeuronCores, data-parallel over words, 2048 words/core):
  - tanh is monotonic => max-pool BEFORE bias+tanh.
  - embedding lookup via one-hot matmul: ids broadcast to 128 partitions,
    VectorE tensor_scalar(is_equal) against a per-partition iota builds the
    one-hot [vocab-half, cols]; two accumulating matmuls against the
    (d-duplicated) embedding table produce x directly as
    [128 partitions (d dup), word-cols] in PSUM. The id plane is DENSE
    (16 cols/word); ScalarE scatters PSUM->SBUF into the strided 21-col
    x-plane (top half as-is, bottom half shifted left one column so a
    single K=128 conv matmul computes TWO taps). Guard cols stay zero
    from a one-time memset.
  - convs packed into 5 PSUM-row tiles so multiple chains share both the
    matmul passes and the reduce:
      A: k6[0:75] + k5[0:53]    3 passes  window [0,21)
      B: k6[75:150] + k5[53:106] 3 passes window [0,21)
      C: k5[106:125] + k4        3 passes window [1,21)
      D: k3 + k1                 2 passes window [0,18)
      E: k2                      1 pass   window [0,17)
    12 passes total (vs 15 for per-chain tiles). Each pass streams 21
    contiguous cols/word; out-of-window cols read only the zero guard so
    they evaluate to exactly 0, making a single full-window reduce_max
    per tile valid (max(y,0) flips only when a word's true max < 0 --
    probability ~2^-16, negligible vs the 2e-2 gate).
  - max over time: ONE VectorE reduce_max per (tile, psum-unit) covering
    all packed rows -> packed feats tiles.
  - ScalarE fused bias+tanh on packed feats; DMA-xbar transpose
    [rows,128]->[128,rows] per word-block; SWDGE cast-DMA (bf16->fp32)
    writes each chain's channel slice straight to DRAM.
"""

import os
import numpy as np
import ml_dtypes

# ---- problem constants (hardcoded; kernel.py must be self-contained) ----
B = 64
WORDS = 256
NW = B * WORDS          # 16384
LMAX = 16
V = 256
D = 64
KS = [1, 2, 3, 4, 5, 6]
CS = [25, 50, 75, 100, 125, 150]
CTOT = sum(CS)          # 525

NCORES = 8
NWC = NW // NCORES      # 2048 words per core
GW = 512                # words per group
NGROUP = NWC // GW      # 4
S = 21                  # word stride in x-plane (16 chars + 5 shared zero pad)
DOFF = 5                # first char col within a word block
NIDXG = ((S * GW + DOFF + 127) // 128) * 128   # 10880 cols per group x-plane
DCOLS = LMAX * GW       # 8192 dense id cols per group
ECH = 512               # embed matmul chunk (one PSUM bank)

OUT_OFF = np.concatenate([[0], np.cumsum(CS)]).tolist()

_BF16 = ml_dtypes.bfloat16

_CACHE = {}

# ---- packed conv tiles -------------------------------------------------
# member: (ki, clo, chi, r0, m)  rows [r0, r0+chi-clo), col-map offset m
# tile: dict(members, sigmas, win0, win)
TILES = [
    dict(members=[(4, 0, 53, 0, 1), (5, 0, 75, 53, 0)],
         sigmas=[-5, -3, -1], win0=0, win=21),
    dict(members=[(4, 53, 106, 0, 1), (5, 75, 150, 53, 0)],
         sigmas=[-5, -3, -1], win0=0, win=21),
    dict(members=[(3, 0, 100, 0, 2), (4, 106, 125, 100, 1)],
         sigmas=[-5, -3, -1], win0=1, win=20),
    dict(members=[(0, 0, 25, 0, 0), (2, 0, 75, 25, 0)],
         sigmas=[-2, 0], win0=0, win=18),
    dict(members=[(1, 0, 50, 0, 0)],
         sigmas=[-1], win0=0, win=17),
]
NTILE = len(TILES)
for _t in TILES:
    _t["rows"] = max(r0 + chi - clo for (_, clo, chi, r0, _) in _t["members"])

# junk cols inside a tile's reduce window for a member whose valid window
# is narrower: (tile -> (row0, row1, jcol0, jcol1)); these evaluate to
# exactly 0 (they read only the zero guard), which would clip the max at 0,
# so a ScalarE splat overwrites them with -1e4 before the reduce.
JUNK = {0: (0, 53, 0, 1), 1: (0, 53, 0, 1),
        2: (0, 100, 1, 2), 3: (0, 25, 16, 18)}

NPASS = sum(len(t["sigmas"]) for t in TILES)   # 12
# wall layout: block i -> cols [128*i, 128*(i+1)), in tile-then-sigma order
_WOFF = {}
_off = 0
for _ti, _t in enumerate(TILES):
    for _s in _t["sigmas"]:
        _WOFF[(_ti, _s)] = _off
        _off += 128
WALL_COLS = _off                               # 12*128 = 1536


def _tap_assign():
    """For each (tile, member, sigma): (dt_top or None, dt_bot or None)."""
    asg = {}
    for ti, t in enumerate(TILES):
        for mi, (ki, clo, chi, r0, m) in enumerate(t["members"]):
            w = KS[ki]
            need = set(range(w))
            for s in t["sigmas"]:
                dt_top = s + m + w - 1
                dt_bot = dt_top + 1
                top = dt_top if dt_top in need else None
                if top is not None:
                    need.discard(dt_top)
                bot = dt_bot if dt_bot in need else None
                if bot is not None:
                    need.discard(dt_bot)
                asg[(ti, mi, s)] = (top, bot)
            assert not need, (ti, mi, need)
    return asg


TAP_ASG = _tap_assign()

# output slices: (tile, col0_in_tr, C, out_off)
OUT_SLICES = []
for _ti, _t in enumerate(TILES):
    for (_ki, _clo, _chi, _r0, _m) in _t["members"]:
        OUT_SLICES.append((_ti, _r0, _chi - _clo, OUT_OFF[_ki] + _clo))


def _build_program():
    from contextlib import ExitStack

    import concourse.mybir as mybir
    import concourse.tile as tile
    from concourse import bacc

    dt = mybir.dt
    nc = bacc.Bacc("TRN2", target_bir_lowering=False, debug=False,
                   num_devices=NCORES)

    idsd = nc.dram_tensor("ids", [1, NGROUP * DCOLS], dt.bfloat16,
                          kind="ExternalInput").ap()
    etab = nc.dram_tensor("etab", [128, 256], dt.bfloat16,
                          kind="ExternalInput").ap()
    iotad = nc.dram_tensor("iota", [128, 2], dt.float32,
                           kind="ExternalInput").ap()
    wall = nc.dram_tensor("wall", [128, WALL_COLS], dt.bfloat16,
                          kind="ExternalInput").ap()
    biasd = nc.dram_tensor("bias", [128, NTILE], dt.float32,
                           kind="ExternalInput").ap()
    fout = nc.dram_tensor("f", [NTILE * 128, NWC], dt.bfloat16,
                          kind="ExternalOutput").ap()

    import concourse.bass as bass

    with tile.TileContext(nc) as tc, ExitStack() as ctx:
        singles = ctx.enter_context(tc.tile_pool(name="singles", bufs=1))
        idsp = ctx.enter_context(tc.tile_pool(name="idsp", bufs=2))
        ohp = ctx.enter_context(tc.tile_pool(name="ohp", bufs=4))
        xpp = ctx.enter_context(tc.tile_pool(name="xpp", bufs=2))
        psp = ctx.enter_context(tc.tile_pool(name="psp", bufs=3, space="PSUM"))
        psep = ctx.enter_context(tc.tile_pool(name="psep", bufs=2,
                                              space="PSUM"))

        etab_sb = singles.tile([128, 256], dt.bfloat16, tag="etab")
        nc.sync.dma_start(out=etab_sb, in_=etab)
        iota_sb = singles.tile([128, 2], dt.float32, tag="iota")
        nc.sync.dma_start(out=iota_sb, in_=iotad)
        wall_sb = singles.tile([128, WALL_COLS], dt.bfloat16, tag="wall")
        nc.sync.dma_start(out=wall_sb, in_=wall)
        bias_sb = singles.tile([128, NTILE], dt.float32, tag="bias")
        nc.sync.dma_start(out=bias_sb, in_=biasd)

        feats = [
            singles.tile([128, NWC], dt.bfloat16, tag=f"feats{i}",
                         name=f"feats{i}")
            for i in range(NTILE)
        ]

        # two persistent x-planes (guard cols stay zero after this memset)
        xps = [singles.tile([128, NIDXG], dt.bfloat16, tag=f"xp{i}",
                            name=f"xp{i}") for i in range(2)]

        def memset_guards():
            # only the inter-word guard cols and the tail need zeroing:
            # top rows need [21n, 21n+5), shifted bottom rows [21n-1, 21n+4)
            for i in range(2):
                xp = xps[i]
                nc.gpsimd.memset(xp[:, 0:DOFF], 0.0)
                nc.gpsimd.memset(
                    xp[:, S - 1: S - 1 + S * (GW - 1)].rearrange(
                        "p (n t) -> p n t", t=S)[:, :, 0:6], 0.0)
                nc.gpsimd.memset(xp[:, S * GW - 1: NIDXG], 0.0)

        def emit_embed_group(g):
            """Embed group g into x-plane xps[g % 2] via one-hot matmuls.
            Returns emission thunks (one per 2-chunk pse pair)."""
            ids_sb = idsp.tile([128, DCOLS], dt.bfloat16, tag="ids",
                               name=f"ids{g}")
            bcast = bass.AP(
                tensor=idsd.tensor,
                offset=g * DCOLS,
                ap=[[0, 128], [1, DCOLS]],
            )
            nc.gpsimd.dma_start(out=ids_sb, in_=bcast)
            xp = xps[g % 2]

            thunks = []
            nch = DCOLS // (4 * ECH)   # 4 thunks of 4*512 dense cols each

            def mk(tci):
                def emit():
                    c0 = tci * 4 * ECH
                    ohs = []
                    for h in range(2):
                        oh = ohp.tile([128, 4 * ECH], dt.bfloat16, tag="oh",
                                      name=f"oh{g}_{tci}_{h}")
                        nc.vector.tensor_scalar(
                            out=oh,
                            in0=ids_sb[:, c0: c0 + 4 * ECH],
                            scalar1=iota_sb[:, h:h + 1],
                            scalar2=None,
                            op0=mybir.AluOpType.is_equal,
                        )
                        ohs.append(oh)
                    for j in range(0, 4 * ECH, ECH):
                        pse = psep.tile([128, ECH], dt.float32, tag="pse",
                                        name=f"pse{g}_{tci}_{j}")
                        for h in range(2):
                            nc.tensor.matmul(
                                pse,
                                lhsT=etab_sb[:, h * 128:(h + 1) * 128],
                                rhs=ohs[h][:, j:j + ECH],
                                start=(h == 0),
                                stop=(h == 1),
                            )
                        # scatter into strided x-plane: 32 words per chunk
                        w0 = (c0 + j) // LMAX
                        nw = ECH // LMAX
                        src = pse.rearrange("p (n t) -> p n t", t=LMAX)
                        top = xp[0:64, S * w0 + DOFF:
                                 S * w0 + DOFF + S * nw].rearrange(
                            "p (n t) -> p n t", t=S)[:, :, 0:LMAX]
                        bot = xp[64:128, S * w0 + DOFF - 1:
                                 S * w0 + DOFF - 1 + S * nw].rearrange(
                            "p (n t) -> p n t", t=S)[:, :, 0:LMAX]
                        nc.scalar.copy(out=top, in_=src[0:64])
                        nc.scalar.copy(out=bot, in_=src[64:128])
                return emit

            for tci in range(nch):
                thunks.append(mk(tci))
            return xp, thunks

        def emit_tile_unit(xp, g, ti, tg):
            """Matmuls + packed reduce for one (conv tile, psum unit)."""
            t = TILES[ti]
            rows = t["rows"]
            ps = psp.tile([128, 2, 512], dt.float32, tag="ps",
                          name=f"ps{g}_{ti}_{tg[0][0]}")
            nsig = len(t["sigmas"])
            win0, win = t["win0"], t["win"]
            # narrow tiles stream only their reduce window's cols per word
            narrow = win < 19
            sw = win if narrow else S
            for si, sg in enumerate(t["sigmas"]):
                boff = _WOFF[(ti, sg)]
                o = DOFF + sg
                for j, (cn0, cnw) in enumerate(tg):
                    rhs = xp[0:128, S * cn0 + o: S * cn0 + o + S * cnw]
                    if narrow:
                        rhs = rhs.rearrange("k (n t) -> k n t",
                                            t=S)[:, :, 0:win]
                    nc.tensor.matmul(
                        ps[:, j, 0:cnw * sw],
                        lhsT=wall_sb[0:128, boff: boff + 128],
                        rhs=rhs,
                        start=(si == 0),
                        stop=(si == nsig - 1),
                    )
            r0 = 0
            while r0 < len(tg):
                r1 = r0
                while r1 < len(tg) and tg[r1][1] == tg[r0][1]:
                    r1 += 1
                na, nwd = r1 - r0, tg[r0][1]
                rw0 = 0 if narrow else win0
                if ti in JUNK:
                    jr0, jr1, jc0, jc1 = JUNK[ti]
                    jv = ps[jr0:jr1, r0:r1, 0:nwd * sw].rearrange(
                        "c a (n t) -> c a n t", t=sw)[:, :, :, jc0:jc1]
                    nc.scalar.activation(
                        out=jv, in_=jv,
                        func=mybir.ActivationFunctionType.Copy,
                        bias=-1.0e4, scale=0.0)
                src = ps[0:rows, r0:r1, 0:nwd * sw].rearrange(
                    "c a (n t) -> c a n t", t=sw)[:, :, :, rw0:rw0 + win]
                w0 = g * GW + tg[r0][0]
                dst = feats[ti][0:rows, w0: w0 + na * nwd].rearrange(
                    "c (a n) -> c a n", n=nwd)
                nc.vector.reduce_max(out=dst, in_=src,
                                     axis=mybir.AxisListType.X)
                r0 = r1

        def emit_output_group(g):
            """bias+tanh per packed tile, then DMA the packed rows out;
            the host unpacks rows->channels and transposes."""
            w0 = g * GW
            for ti, t in enumerate(TILES):
                nc.scalar.activation(
                    out=feats[ti][0:t["rows"], w0:w0 + GW],
                    in_=feats[ti][0:t["rows"], w0:w0 + GW],
                    func=mybir.ActivationFunctionType.Tanh,
                    bias=bias_sb[0:t["rows"], ti:ti + 1],
                )
                nc.sync.dma_start(
                    out=fout[ti * 128: ti * 128 + t["rows"], w0:w0 + GW],
                    in_=feats[ti][0:t["rows"], w0:w0 + GW])

        # psum units: chunks of 24 words (504 cols), paired into 2-bank units
        wpb = 512 // S
        chunks = []
        n0 = 0
        while n0 < GW:
            rem = GW - n0
            cw = wpb if rem > 32 else (16 if rem == 32 else rem)
            chunks.append((n0, cw))
            n0 += cw
        units = [tuple(chunks[i:i + 2]) for i in range(0, len(chunks), 2)]

        # group-0 embed thunks are emitted lazily, right before the first
        # conv unit that needs their words (thunk i covers words 32i..32i+32)
        xp_cur, cthunks = emit_embed_group(0)   # issues g0 ids DMA first
        memset_guards()
        cdone = 0

        for g in range(NGROUP):
            items = [(ti, tg) for tg in units for ti in range(NTILE)]
            if g + 1 < NGROUP:
                xp_next, nthunks = emit_embed_group(g + 1)
            else:
                xp_next, nthunks = None, []
            ne, ni = len(nthunks), len(items)
            ti_ = 0
            for k, (ti, tg) in enumerate(items):
                wend = tg[-1][0] + tg[-1][1]      # last word this unit reads
                need = min(len(cthunks), -(-wend // 128))
                while cdone < need:
                    cthunks[cdone]()
                    cdone += 1
                emit_tile_unit(xp_cur, g, ti, tg)
                want = (k + 1) * ne // ni
                while ti_ < want:
                    nthunks[ti_]()
                    ti_ += 1
            while cdone < len(cthunks):
                cthunks[cdone]()
                cdone += 1
            while ti_ < ne:
                nthunks[ti_]()
                ti_ += 1
            if g > 0:
                emit_output_group(g - 1)
            xp_cur = xp_next
            cthunks, cdone = nthunks, len(nthunks)
        emit_output_group(NGROUP - 1)

    nc.compile()
    return nc


def _host_consts(emb, Ws, bs):
    e = emb.astype(_BF16)
    etab = np.zeros((128, 256), dtype=_BF16)
    for h in range(2):
        etab[:, h * 128: h * 128 + 64] = e[h * 128:(h + 1) * 128, :]
        etab[:, h * 128 + 64: h * 128 + 128] = e[h * 128:(h + 1) * 128, :]

    iota = np.zeros((128, 2), dtype=np.float32)
    iota[:, 0] = np.arange(128)
    iota[:, 1] = np.arange(128, 256)

    wall = np.zeros((128, WALL_COLS), dtype=_BF16)
    for ti, t in enumerate(TILES):
        for mi, (ki, clo, chi, r0, m) in enumerate(t["members"]):
            Wb = Ws[ki].astype(np.float32)
            c = chi - clo
            for sg in t["sigmas"]:
                boff = _WOFF[(ti, sg)]
                dt_top, dt_bot = TAP_ASG[(ti, mi, sg)]
                if dt_top is not None:
                    wall[0:64, boff + r0: boff + r0 + c] = (
                        Wb[clo:chi, :, dt_top].T.astype(_BF16))
                if dt_bot is not None:
                    wall[64:128, boff + r0: boff + r0 + c] = (
                        Wb[clo:chi, :, dt_bot].T.astype(_BF16))

    bias = np.zeros((128, NTILE), dtype=np.float32)
    for ti, t in enumerate(TILES):
        for (ki, clo, chi, r0, m) in t["members"]:
            bias[r0: r0 + chi - clo, ti] = bs[ki][clo:chi]
    return etab, iota, wall, bias


def _host_ids(char_ids_core):
    """Per-core dense id plane [1, NGROUP*DCOLS] bf16."""
    return (char_ids_core.astype(np.float32)
            .reshape(1, -1).astype(_BF16))


def kernel(**inputs):
    import jax

    jax.devices()  # boot the axon PJRT backend
    from concourse.bass_utils import run_bass_kernel_spmd

    char_ids = np.asarray(inputs["char_ids"], dtype=np.int32)
    word_pos = np.asarray(inputs["word_pos"], dtype=np.int64)
    word_batch = np.asarray(inputs["word_batch"], dtype=np.int64)
    emb = np.asarray(inputs["emb"], dtype=np.float32)
    Ws = [np.asarray(inputs[f"W{i+1}"], dtype=np.float32) for i in range(6)]
    bs = [np.asarray(inputs[f"b{i+1}"], dtype=np.float32) for i in range(6)]

    if "nc" not in _CACHE:
        _CACHE["nc"] = _build_program()
    nc = _CACHE["nc"]

    etab, iota, wall, bias = _host_consts(emb, Ws, bs)
    in_maps = []
    for c in range(NCORES):
        in_maps.append({
            "ids": _host_ids(char_ids[c * NWC:(c + 1) * NWC]),
            "etab": etab,
            "iota": iota,
            "wall": wall,
            "bias": bias,
        })

    core_ids = list(range(NCORES))
    trace = bool(os.environ.get("KERNEL_TRACE"))
    res = run_bass_kernel_spmd(nc, in_maps, core_ids, trace=trace)
    if trace:
        _CACHE["last_exec_time_ns"] = res.exec_time_ns

    f_full = np.empty((NW, CTOT), dtype=np.float32)
    for c in core_ids:
        fb = np.asarray(res.results[c]["f"])      # [NTILE*128, NWC] bf16
        dst = f_full[c * NWC:(c + 1) * NWC]
        for (ti, r0, C, oo) in OUT_SLICES:
            dst[:, oo:oo + C] = fb[ti * 128 + r0: ti * 128 + r0 + C, :].T

    out = np.zeros((WORDS, B, CTOT), dtype=np.float32)
    out[word_pos, word_batch] = f_full
    return out
